# revision 22
# baseline (speedup 1.0000x reference)
"""Trainium2 Bass kernel for the PINN-style loss problem (v2).

Math: a 6-layer tanh MLP u(x,t) (2->50x5->1) is evaluated with forward-mode
jets (u, u_x, u_t, u_xxx) at N=10000 points. The per-param loss
  loss_p = mean_n (u_t + a_p*u*u_x + b_p*u_xxx + c_p*u_x)^2
collapses to loss_p = ptilde^T G ptilde / N with ptilde = [a,b,c,1] and G the
4x4 Gram of g_n = [u*u_x, u_xxx, u_x, u_t].

v2 design (vs v1):
- No collective. Each core evaluates the tower on its 1250-point x-shard,
  builds its partial Gram G_c, and computes partial losses for ALL 5000
  params q_c[p] = ptilde_p^T G_c ptilde_p / N via one block-diagonal matmul
  against a host-precomputed monomial tensor. The host sums the 8 partial
  loss vectors (loss is linear in G). This removes the AllReduce (9-13us)
  and the old 19us post-AR tail.
- fp16 streams + fp16 matmuls (1 cyc/col at any width; no f32r <256-col
  penalty), DVE 2-byte fast modes for elementwise.
- FD=625 per block (1250 = 2x625): no padded points, no masking.
- Gram via PE transpose of the projected [8,625] stream rows instead of
  20 stationary-stream matmuls.
"""

import os
import sys
import numpy as np

for _p in ("/opt/trn_rl_repo",):
    if os.path.isdir(_p) and _p not in sys.path:
        sys.path.append(_p)

import concourse.bass as bass
import concourse.bacc as bacc
import concourse.mybir as mybir
import concourse.tile as tile
from concourse import bass_utils

F32 = mybir.dt.float32
F16 = mybir.dt.float16
AF = mybir.ActivationFunctionType
ALU = mybir.AluOpType

NCORES = 8
NPTS = 10000
NPC = NPTS // NCORES       # 1250 points per core
FD = 625                   # free dim per block: 2 blocks x 625 = 1250, no pad
HB = 100                   # 2 blocks x 50 hidden units
CHUNKS = ((0, 512), (512, FD - 512))   # matmul free-dim chunks (psum bank)
PG = 4                     # para groups (blockdiag K=128 = 4 groups x 32 rows)
PPG = 5000 // PG           # 1250 paras per group
FDP = 1024                 # padded tile free dim (2KB fp16): sbuf bank stride
GS = 1e-2                  # Gram-side scale (1/N split as GS*GS_mono)

F32R = mybir.dt.float32r
SDT = F32R if os.environ.get("KSDT") == "f32r" else F16   # stream/weight dtype
NPDT = np.float32 if os.environ.get("KSDT") == "f32r" else np.float16


def _t(pool, tag, dt=None):
    d = dt if dt is not None else SDT
    return pool.tile([HB, FD], d, tag=tag, name=tag,
                     padded_shape=[HB, FDP])


def _mm(nc, out, lhsT, rhs, start=True, stop=True):
    nc.tensor.matmul(out, lhsT, rhs, start=start, stop=stop)


def _mm_chunks(nc, out_tile, lhsT, rhs_tile, chunks=CHUNKS):
    for off, w in chunks:
        _mm(nc, out_tile[:, off:off + w], lhsT, rhs_tile[:, off:off + w])


def build_program(stage="full"):
    nc = bacc.Bacc("TRN2", target_bir_lowering=False, debug=False)

    h0_d = nc.dram_tensor("h0", [4, FD], SDT, kind="ExternalInput")
    w1t_d = nc.dram_tensor("w1t", [4, HB], SDT, kind="ExternalInput")
    wb_d = nc.dram_tensor("wb", [HB, 400], SDT, kind="ExternalInput")
    w6p_d = nc.dram_tensor("w6p", [HB, 2], SDT, kind="ExternalInput")
    vecs_d = nc.dram_tensor("vecs", [HB, 11], F32, kind="ExternalInput")
    iden_d = nc.dram_tensor("iden8", [8, 8], SDT, kind="ExternalInput")
    mono_d = nc.dram_tensor("mono", [128, PPG], SDT, kind="ExternalInput")
    if stage == "tower":
        loss_d = nc.dram_tensor("dbg", [HB, FD], F32, kind="ExternalOutput")
    elif stage == "gram":
        loss_d = nc.dram_tensor("dbg", [4, 4], F32, kind="ExternalOutput")
    else:
        loss_d = nc.dram_tensor("loss", [PG, PPG], F32, kind="ExternalOutput")

    with tile.TileContext(nc) as tc:
        _body(tc, nc, h0_d, w1t_d, wb_d, w6p_d, vecs_d, iden_d, mono_d,
              loss_d, stage=stage)
    nc.compile()
    return nc


def _body(tc, nc, h0_d, w1t_d, wb_d, w6p_d, vecs_d, iden_d, mono_d, loss_d,
          stage="full"):
    import contextlib

    ctx = contextlib.ExitStack()
    with ctx:
        cpool = ctx.enter_context(tc.tile_pool(name="const", bufs=1))
        spool = ctx.enter_context(tc.tile_pool(name="streams", bufs=2))
        tpool = ctx.enter_context(tc.tile_pool(name="trans", bufs=2))
        dpool = ctx.enter_context(tc.tile_pool(name="dram", bufs=1, space="DRAM"))

        # ---- load constants (split across engine DMA queues) ----
        h0 = cpool.tile([4, FD], SDT, tag="h0")
        w1t = cpool.tile([4, HB], SDT, tag="w1t")
        wb = cpool.tile([HB, 400], SDT, tag="wb")
        w6p = cpool.tile([HB, 2], SDT, tag="w6p")
        vecs = cpool.tile([HB, 11], F32, tag="vecs")
        iden8 = cpool.tile([8, 8], SDT, tag="iden8")
        mono = cpool.tile([128, PPG], SDT, tag="mono")
        gBD = cpool.tile([128, PG], SDT, tag="gBD")

        nc.sync.dma_start(h0[:], h0_d[:])
        nc.sync.dma_start(w1t[:], w1t_d[:])
        nc.scalar.dma_start(vecs[:], vecs_d[:])
        nc.scalar.dma_start(wb[:], wb_d[:])
        nc.gpsimd.dma_start(mono[:], mono_d[:])
        nc.gpsimd.dma_start(w6p[:], w6p_d[:])
        nc.gpsimd.dma_start(iden8[:], iden_d[:])
        nc.gpsimd.memset(gBD[:], 0.0)
        # DVE warmup: the DVE ramps to full clock with a busy streak; burn
        # idle startup time so layer-1 ops run at full speed.
        warm = cpool.tile([HB, FD], SDT, tag="warm")
        nc.vector.memset(warm[:], 1.0)
        for _ in range(4):
            nc.vector.tensor_scalar(warm[:], warm[:], 1.0001, None, ALU.mult)

        cx = vecs[:, 0:1]
        ct = vecs[:, 1:2]
        cm2x2 = vecs[:, 2:3]
        cx3 = vecs[:, 3:4]

        def bb(layer):  # bias vector for layer 1..5
            return vecs[:, 3 + layer:4 + layer]

        neg2 = vecs[:, 9:10]
        b6sc = vecs[0:2, 10:11]     # layer-6 bias replicated on 2 partitions

        v = nc.vector
        s = nc.scalar
        g = nc.gpsimd

        with tc.tile_pool(name="ztw", bufs=3, space="PSUM") as zpool:
            # ---------- layer 1 ----------
            z = zpool.tile([HB, FD], F32, tag="ztw")
            _mm_chunks(nc, z, w1t[:], h0)
            a = _t(spool, "a")
            s.activation(a[:], z[:], AF.Tanh, bias=bb(1))
            asq = _t(tpool, "asq")
            s.activation(asq[:], a[:], AF.Square)
            f1 = _t(tpool, "f1")
            v.tensor_scalar(f1[:], asq[:], -1.0, 1.0, ALU.mult, ALU.add)
            h6 = _t(tpool, "h6")
            v.tensor_scalar(h6[:], asq[:], 6.0, -2.0, ALU.mult, ALU.add)
            ax = _t(spool, "ax")
            v.tensor_scalar(ax[:], f1[:], cx, None, ALU.mult)
            at = _t(spool, "at")
            s.activation(at[:], f1[:], AF.Identity, scale=ct)
            af1 = _t(tpool, "p1")
            g.tensor_tensor(af1[:], a[:], f1[:], ALU.mult)
            axx = _t(spool, "axx")
            s.activation(axx[:], af1[:], AF.Identity, scale=cm2x2)
            f3 = _t(tpool, "p2")
            g.tensor_tensor(f3[:], f1[:], h6[:], ALU.mult)
            axxx = _t(spool, "axxx")
            s.activation(axxx[:], f3[:], AF.Identity, scale=cx3)

            # ---------- layers 2..5 ----------
            for layer in range(2, 6):
                W = wb[:, 100 * (layer - 2):100 * (layer - 1)]
                last = layer == 5

                z = zpool.tile([HB, FD], F32, tag="ztw")
                _mm_chunks(nc, z, W, a)
                a_n = _t(spool, "a")
                s.activation(a_n[:], z[:], AF.Tanh, bias=bb(layer))

                zx = zpool.tile([HB, FD], F32, tag="ztw")
                _mm_chunks(nc, zx, W, ax)
                # zx copy to SBUF fp16 (consumed 3-4x by DVE fast-mode ops)
                zxC = tpool.tile([HB, FD], SDT, tag="zxC")
                s.activation(zxC[:], zx[:], AF.Copy)
                asq = _t(tpool, "asq")
                s.activation(asq[:], a_n[:], AF.Square)
                f1 = _t(tpool, "f1")
                v.tensor_scalar(f1[:], asq[:], -1.0, 1.0, ALU.mult, ALU.add)
                ax_n = _t(spool, "ax")
                v.tensor_tensor(ax_n[:], f1[:], zxC[:], ALU.mult)
                w2 = _t(tpool, "w2")
                s.activation(w2[:], zxC[:], AF.Square)

                zt = zpool.tile([HB, FD], F32, tag="ztw")
                _mm_chunks(nc, zt, W, at)
                at_n = _t(spool, "at")
                v.tensor_tensor(at_n[:], f1[:], zt[:], ALU.mult)

                zxx = zpool.tile([HB, FD], F32, tag="ztw")
                _mm_chunks(nc, zxx, W, axx)
                zxxC = tpool.tile([HB, FD], SDT, tag="zxxC")
                s.activation(zxxC[:], zxx[:], AF.Copy)
                h6 = _t(tpool, "h6")
                v.tensor_scalar(h6[:], asq[:], 6.0, -2.0, ALU.mult, ALU.add)
                P = _t(tpool, "p1")
                g.tensor_tensor(P[:], a_n[:], zxC[:], ALU.mult)
                zx3 = _t(tpool, "zx3")
                g.tensor_tensor(zx3[:], w2[:], zxC[:], ALU.mult)

                zxxx = zpool.tile([HB, FD], F32, tag="ztw")
                _mm_chunks(nc, zxxx, W, axxx)
                if not last:
                    gt = _t(tpool, "g")
                    g.tensor_tensor(gt[:], a_n[:], w2[:], ALU.mult)
                    inner = _t(tpool, "inner")
                    v.scalar_tensor_tensor(inner[:], gt[:], -2.0, zxxC[:],
                                           ALU.mult, ALU.add)
                m = _t(tpool, "p2")
                v.tensor_tensor(m[:], P[:], zxxC[:], ALU.mult)
                if not last:
                    axx_n = _t(spool, "axx")
                    v.tensor_tensor(axx_n[:], f1[:], inner[:], ALU.mult)

                i3a = _t(tpool, "i3a")
                v.scalar_tensor_tensor(i3a[:], m[:], -6.0, zxxx[:],
                                       ALU.mult, ALU.add)
                n_t = _t(tpool, "n")
                v.tensor_tensor(n_t[:], h6[:], zx3[:], ALU.mult)
                i3 = _t(tpool, "i3")
                v.tensor_tensor(i3[:], i3a[:], n_t[:], ALU.add)
                axxx_n = _t(spool, "axxx")
                v.tensor_tensor(axxx_n[:], f1[:], i3[:], ALU.mult)

                a, at, ax, axxx = a_n, at_n, ax_n, axxx_n
                if not last:
                    axx = axx_n

            a5, ax5, at5, axxx5 = a, ax, at, axxx

        if stage == "tower":
            nc.sync.dma_start(loss_d[:], axxx5[:].bitcast(F32))
            return

        # ---------- layer 6 projection + Gram ----------
        # U8 rows (2s+b): s=0 u, 1 uxxx, 2 ux, 3 ut; b = block.
        with tc.tile_pool(name="proj", bufs=2, space="PSUM") as ppool:
            # pair tiles [2,FD] fp16, rows = (block0, block1):
            # puux = (u+b6)*ux, puxxx, pux, put. All partition-0 based.
            # U2 slots rotate (bufs=2); order keeps WAR deps acyclic.
            Ux = ppool.tile([2, FD], F32, tag="U2")
            _mm_chunks(nc, Ux, w6p[:], ax5[:])
            pux = cpool.tile([2, FD], SDT, tag="pux")
            s.activation(pux[:], Ux[:], AF.Copy)
            Uu = ppool.tile([2, FD], F32, tag="U2")
            _mm_chunks(nc, Uu, w6p[:], a5[:])
            puux = cpool.tile([2, FD], SDT, tag="puux")
            v.scalar_tensor_tensor(puux[:], Uu[:], b6sc, pux[:],
                                   ALU.add, ALU.mult)
            Ut = ppool.tile([2, FD], F32, tag="U2")
            _mm_chunks(nc, Ut, w6p[:], at5[:])
            put = cpool.tile([2, FD], SDT, tag="put")
            s.activation(put[:], Ut[:], AF.Copy)
            Uxxx = ppool.tile([2, FD], F32, tag="U2")
            _mm_chunks(nc, Uxxx, w6p[:], axxx5[:])
            puxxx = cpool.tile([2, FD], SDT, tag="puxxx")
            s.activation(puxxx[:], Uxxx[:], AF.Copy)
            pairs = (puux, puxxx, pux, put)

        with tc.tile_pool(name="psmall", bufs=1, space="PSUM") as pps:
            G4 = pps.tile([4, 4], F32, tag="G4")
            iden2 = iden8[0:2, 0:2]
            TCH = ((0, 128), (128, 128), (256, 128), (384, 128), (512, 113))
            for c, (lo, w) in enumerate(TCH):
                chT_p = pps.tile([128, 8], SDT, tag="chT")
                for sidx, pair in enumerate(pairs):
                    nc.tensor.transpose(chT_p[0:w, 2 * sidx:2 * sidx + 2],
                                        pair[:, lo:lo + w], iden2)
                chT = tpool.tile([128, 8], SDT, tag="chTs")
                v.tensor_copy(chT[0:w, :], chT_p[0:w, :])
                chv = chT[0:w, :].rearrange("p (s b) -> p b s", b=2, s=4)
                for b in range(2):
                    _mm(nc, G4[:], chv[:, b, :], chv[:, b, :],
                        start=(c == 0 and b == 0), stop=(c == 4 and b == 1))

            g16f = cpool.tile([4, 4], SDT, tag="g16f")
            s.activation(g16f[:], G4[:], AF.Copy, scale=GS)

            if stage == "gram":
                gg = cpool.tile([4, 4], F32, tag="gg")
                v.tensor_copy(gg[:], G4[:])
                nc.sync.dma_start(loss_d[:], gg[:])
                return

            # bounce g16 through DRAM once, then DVE-replicate to the
            # 32-aligned diagonal blocks (start partitions 0/32/64/96).
            g16d = dpool.tile([1, 16], SDT, tag="g16d")
            nc.sync.dma_start(g16d[:], g16f[:])
            nc.sync.dma_start(gBD[0:16, 0:1], g16d[:])
            for bidx in range(1, PG):
                v.tensor_copy(gBD[32 * bidx:32 * bidx + 16, bidx:bidx + 1],
                              gBD[0:16, 0:1])

            # ---------- partial losses for all 5000 paras ----------
            loss8 = pps.tile([PG, PPG], F32, tag="loss8")
            PCH = ((0, 512), (512, 512), (1024, PPG - 1024))
            _mm_chunks(nc, loss8, gBD[:], mono[:], chunks=PCH)
            lossS = cpool.tile([PG, PPG], F32, tag="lossS")
            s.activation(lossS[:], loss8[:], AF.Copy)
            nc.sync.dma_start(loss_d[:], lossS[:])


def prep_inputs(x, para, W1, b1, W2, b2, W3, b3, W4, b4, W5, b5, W6, b6):
    """Full inputs -> list of per-core input dicts (host-side shard/layout)."""
    f = np.float32
    h = NPDT
    x = np.asarray(x, f)
    para = np.asarray(para, f)
    Ws = [np.asarray(W, f) for W in (W1, W2, W3, W4, W5, W6)]
    bs = [np.asarray(b, f) for b in (b1, b2, b3, b4, b5, b6)]

    w1t = np.zeros((4, HB), h)
    w1t[0:2, 0:50] = Ws[0].T
    w1t[2:4, 50:100] = Ws[0].T
    wb = np.zeros((HB, 400), h)
    for i in range(4):
        W = Ws[i + 1]
        wb[0:50, 100 * i:100 * i + 50] = W.T
        wb[50:100, 100 * i + 50:100 * i + 100] = W.T
    w6p = np.zeros((HB, 2), h)
    w6p[0:50, 0] = Ws[5][0]
    w6p[50:100, 1] = Ws[5][0]
    vecs = np.zeros((HB, 11), f)
    vecs[:, 10] = bs[5][0]
    vecs[:, 9] = -2.0
    cx = Ws[0][:, 0]
    ct = Ws[0][:, 1]
    for half in (slice(0, 50), slice(50, 100)):
        vecs[half, 0] = cx
        vecs[half, 1] = ct
        vecs[half, 2] = -2.0 * cx * cx
        vecs[half, 3] = cx * cx * cx
        for l in range(5):
            vecs[half, 4 + l] = bs[l]
    iden8 = np.eye(8, dtype=h)

    # mono[16*b + 4*i + j, k] = ptilde_i * ptilde_j * GS for para[625*b + k]
    pt = np.concatenate([para, np.ones((5000, 1), f)], axis=1)  # [5000,4]
    mono_full = (pt[:, :, None] * pt[:, None, :] * GS).reshape(5000, 16)
    mono = np.zeros((128, PPG), h)
    for b in range(PG):
        mono[32 * b:32 * b + 16, :] = mono_full[PPG * b:PPG * (b + 1)].T

    maps = []
    for c in range(NCORES):
        sl = x[c * NPC:(c + 1) * NPC]
        h0 = np.zeros((4, FD), h)
        h0[0] = sl[0:FD, 0]
        h0[1] = sl[0:FD, 1]
        h0[2] = sl[FD:NPC, 0]
        h0[3] = sl[FD:NPC, 1]
        maps.append({
            "h0": h0, "w1t": w1t, "wb": wb, "w6p": w6p, "vecs": vecs,
            "iden8": iden8, "mono": mono,
        })
    return maps


_NC_CACHE = {}


def get_program():
    if "nc" not in _NC_CACHE:
        _NC_CACHE["nc"] = build_program()
    return _NC_CACHE["nc"]


def kernel(x, para, W1, b1, W2, b2, W3, b3, W4, b4, W5, b5, W6, b6):
    maps = prep_inputs(x, para, W1, b1, W2, b2, W3, b3, W4, b4, W5, b5, W6, b6)
    nc = get_program()
    res = bass_utils.run_bass_kernel_spmd(nc, maps, list(range(NCORES)))
    out = np.zeros(5000, np.float64)
    for c in range(NCORES):
        out += res.results[c]["loss"].astype(np.float64).reshape(-1)
    return out.astype(np.float32)


# revision 25
# speedup vs baseline: 1.0550x; 1.0550x over previous
"""Trainium2 Bass kernel for the PINN-style loss problem (v2).

Math: a 6-layer tanh MLP u(x,t) (2->50x5->1) is evaluated with forward-mode
jets (u, u_x, u_t, u_xxx) at N=10000 points. The per-param loss
  loss_p = mean_n (u_t + a_p*u*u_x + b_p*u_xxx + c_p*u_x)^2
collapses to loss_p = ptilde^T G ptilde / N with ptilde = [a,b,c,1] and G the
4x4 Gram of g_n = [u*u_x, u_xxx, u_x, u_t].

v2 design (vs v1):
- No collective. Each core evaluates the tower on its 1250-point x-shard,
  builds its partial Gram G_c, and computes partial losses for ALL 5000
  params q_c[p] = ptilde_p^T G_c ptilde_p / N via one block-diagonal matmul
  against a host-precomputed monomial tensor. The host sums the 8 partial
  loss vectors (loss is linear in G). This removes the AllReduce (9-13us)
  and the old 19us post-AR tail.
- fp16 streams + fp16 matmuls (1 cyc/col at any width; no f32r <256-col
  penalty), DVE 2-byte fast modes for elementwise.
- FD=625 per block (1250 = 2x625): no padded points, no masking.
- Gram via PE transpose of the projected [8,625] stream rows instead of
  20 stationary-stream matmuls.
"""

import os
import sys
import numpy as np

for _p in ("/opt/trn_rl_repo",):
    if os.path.isdir(_p) and _p not in sys.path:
        sys.path.append(_p)

import concourse.bass as bass
import concourse.bacc as bacc
import concourse.mybir as mybir
import concourse.tile as tile
from concourse import bass_utils

F32 = mybir.dt.float32
F16 = mybir.dt.float16
AF = mybir.ActivationFunctionType
ALU = mybir.AluOpType

NCORES = 8
NPTS = 10000
NPC = NPTS // NCORES       # 1250 points per core
FD = 625                   # free dim per block: 2 blocks x 625 = 1250, no pad
HB = 100                   # 2 blocks x 50 hidden units
CHUNKS = ((0, 512), (512, FD - 512))   # matmul free-dim chunks (psum bank)
PG = 4                     # para groups (blockdiag K=128 = 4 groups x 32 rows)
PPG = 5000 // PG           # 1250 paras per group
FDP = 1024                 # padded tile free dim (2KB fp16): sbuf bank stride
GS = 1e-2                  # Gram-side scale (1/N split as GS*GS_mono)

F32R = mybir.dt.float32r
SDT = F32R if os.environ.get("KSDT") == "f32r" else F16   # stream/weight dtype
NPDT = np.float32 if os.environ.get("KSDT") == "f32r" else np.float16


def _t(pool, tag, dt=None):
    d = dt if dt is not None else SDT
    return pool.tile([HB, FD], d, tag=tag, name=tag,
                     padded_shape=[HB, FDP])


def _mm(nc, out, lhsT, rhs, start=True, stop=True):
    nc.tensor.matmul(out, lhsT, rhs, start=start, stop=stop)


def _mm_chunks(nc, out_tile, lhsT, rhs_tile, chunks=CHUNKS):
    for off, w in chunks:
        _mm(nc, out_tile[:, off:off + w], lhsT, rhs_tile[:, off:off + w])


def build_program(stage="full"):
    nc = bacc.Bacc("TRN2", target_bir_lowering=False, debug=False)

    h0_d = nc.dram_tensor("h0", [4, FD], SDT, kind="ExternalInput")
    w1t_d = nc.dram_tensor("w1t", [4, HB], SDT, kind="ExternalInput")
    wb_d = nc.dram_tensor("wb", [HB, 400], SDT, kind="ExternalInput")
    w6p_d = nc.dram_tensor("w6p", [HB, 2], SDT, kind="ExternalInput")
    vecs_d = nc.dram_tensor("vecs", [HB, 11], F32, kind="ExternalInput")
    iden_d = nc.dram_tensor("iden8", [8, 8], SDT, kind="ExternalInput")
    mono_d = nc.dram_tensor("mono", [128, PPG], SDT, kind="ExternalInput")
    if stage == "tower":
        loss_d = nc.dram_tensor("dbg", [HB, FD], F32, kind="ExternalOutput")
    elif stage == "gram":
        loss_d = nc.dram_tensor("dbg", [4, 4], F32, kind="ExternalOutput")
    else:
        loss_d = nc.dram_tensor("loss", [PG, PPG], F32, kind="ExternalOutput")

    with tile.TileContext(nc) as tc:
        _body(tc, nc, h0_d, w1t_d, wb_d, w6p_d, vecs_d, iden_d, mono_d,
              loss_d, stage=stage)
    nc.compile()
    return nc


def _body(tc, nc, h0_d, w1t_d, wb_d, w6p_d, vecs_d, iden_d, mono_d, loss_d,
          stage="full"):
    import contextlib

    ctx = contextlib.ExitStack()
    with ctx:
        cpool = ctx.enter_context(tc.tile_pool(name="const", bufs=1))
        spool = ctx.enter_context(tc.tile_pool(name="streams", bufs=2))
        tpool = ctx.enter_context(tc.tile_pool(name="trans", bufs=2))
        dpool = ctx.enter_context(tc.tile_pool(name="dram", bufs=1, space="DRAM"))

        # ---- load constants (split across engine DMA queues) ----
        h0 = cpool.tile([4, FD], SDT, tag="h0")
        w1t = cpool.tile([4, HB], SDT, tag="w1t")
        wb = cpool.tile([HB, 400], SDT, tag="wb")
        w6p = cpool.tile([HB, 2], SDT, tag="w6p")
        vecs = cpool.tile([HB, 11], F32, tag="vecs")
        iden8 = cpool.tile([8, 8], SDT, tag="iden8")
        mono = cpool.tile([128, PPG], SDT, tag="mono")
        gBD = cpool.tile([128, PG], SDT, tag="gBD")

        nc.sync.dma_start(h0[:], h0_d[:])
        nc.sync.dma_start(w1t[:], w1t_d[:])
        nc.scalar.dma_start(vecs[:], vecs_d[:])
        nc.scalar.dma_start(wb[:], wb_d[:])
        nc.gpsimd.dma_start(mono[:], mono_d[:])
        nc.gpsimd.dma_start(w6p[:], w6p_d[:])
        nc.gpsimd.dma_start(iden8[:], iden_d[:])
        nc.gpsimd.memset(gBD[:], 0.0)
        # DVE warmup: the DVE ramps to full clock with a busy streak; burn
        # idle startup time so layer-1 ops run at full speed.
        warm = cpool.tile([HB, FD], SDT, tag="warm")
        nc.vector.memset(warm[:], 1.0)
        for _ in range(4):
            nc.vector.tensor_scalar(warm[:], warm[:], 1.0001, None, ALU.mult)

        cx = vecs[:, 0:1]
        ct = vecs[:, 1:2]
        cm2x2 = vecs[:, 2:3]
        cx3 = vecs[:, 3:4]

        def bb(layer):  # bias vector for layer 1..5
            return vecs[:, 3 + layer:4 + layer]

        neg2 = vecs[:, 9:10]
        b6sc = vecs[0:2, 10:11]     # layer-6 bias replicated on 2 partitions

        v = nc.vector
        s = nc.scalar
        g = nc.gpsimd

        with tc.tile_pool(name="ztw", bufs=3, space="PSUM") as zpool:
            # ---------- layer 1 ----------
            z = zpool.tile([HB, FD], F32, tag="ztw")
            _mm_chunks(nc, z, w1t[:], h0)
            a = _t(spool, "a")
            s.activation(a[:], z[:], AF.Tanh, bias=bb(1))
            asq = _t(tpool, "asq")
            s.activation(asq[:], a[:], AF.Square)
            f1 = _t(tpool, "f1")
            v.tensor_scalar(f1[:], asq[:], -1.0, 1.0, ALU.mult, ALU.add)
            h6 = _t(tpool, "h6")
            v.tensor_scalar(h6[:], asq[:], 6.0, -2.0, ALU.mult, ALU.add)
            ax = _t(spool, "ax")
            v.tensor_scalar(ax[:], f1[:], cx, None, ALU.mult)
            at = _t(spool, "at")
            s.activation(at[:], f1[:], AF.Identity, scale=ct)
            af1 = _t(tpool, "p1")
            g.tensor_tensor(af1[:], a[:], f1[:], ALU.mult)
            axx = _t(spool, "axx")
            s.activation(axx[:], af1[:], AF.Identity, scale=cm2x2)
            f3 = _t(tpool, "p2")
            g.tensor_tensor(f3[:], f1[:], h6[:], ALU.mult)
            axxx = _t(spool, "axxx")
            s.activation(axxx[:], f3[:], AF.Identity, scale=cx3)

            # ---------- layers 2..5 ----------
            for layer in range(2, 6):
                W = wb[:, 100 * (layer - 2):100 * (layer - 1)]
                last = layer == 5

                z = zpool.tile([HB, FD], F32, tag="ztw")
                _mm_chunks(nc, z, W, a)
                a_n = _t(spool, "a")
                s.activation(a_n[:], z[:], AF.Tanh, bias=bb(layer))

                zx = zpool.tile([HB, FD], F32, tag="ztw")
                _mm_chunks(nc, zx, W, ax)
                # zx copy to SBUF fp16 (consumed 3-4x by DVE fast-mode ops)
                zxC = tpool.tile([HB, FD], SDT, tag="zxC")
                s.activation(zxC[:], zx[:], AF.Copy)
                asq = _t(tpool, "asq")
                s.activation(asq[:], a_n[:], AF.Square)
                f1 = _t(tpool, "f1")
                v.tensor_scalar(f1[:], asq[:], -1.0, 1.0, ALU.mult, ALU.add)
                ax_n = _t(spool, "ax")
                g.tensor_tensor(ax_n[:], f1[:], zxC[:], ALU.mult)
                w2 = _t(tpool, "w2")
                s.activation(w2[:], zxC[:], AF.Square)

                zt = zpool.tile([HB, FD], F32, tag="ztw")
                _mm_chunks(nc, zt, W, at)
                at_n = _t(spool, "at")
                v.tensor_tensor(at_n[:], f1[:], zt[:], ALU.mult)

                zxx = zpool.tile([HB, FD], F32, tag="ztw")
                _mm_chunks(nc, zxx, W, axx)
                zxxC = tpool.tile([HB, FD], SDT, tag="zxxC")
                s.activation(zxxC[:], zxx[:], AF.Copy)
                h6 = _t(tpool, "h6")
                v.tensor_scalar(h6[:], asq[:], 6.0, -2.0, ALU.mult, ALU.add)
                P = _t(tpool, "p1")
                v.tensor_tensor(P[:], a_n[:], zxC[:], ALU.mult)
                zx3 = _t(tpool, "zx3")
                v.tensor_tensor(zx3[:], w2[:], zxC[:], ALU.mult)

                zxxx = zpool.tile([HB, FD], F32, tag="ztw")
                _mm_chunks(nc, zxxx, W, axxx)
                if not last:
                    gt = _t(tpool, "g")
                    v.tensor_tensor(gt[:], a_n[:], w2[:], ALU.mult)
                    inner = _t(tpool, "inner")
                    v.scalar_tensor_tensor(inner[:], gt[:], -2.0, zxxC[:],
                                           ALU.mult, ALU.add)
                m = _t(tpool, "p2")
                v.tensor_tensor(m[:], P[:], zxxC[:], ALU.mult)
                if not last:
                    axx_n = _t(spool, "axx")
                    g.tensor_tensor(axx_n[:], f1[:], inner[:], ALU.mult)

                i3a = _t(tpool, "i3a")
                v.scalar_tensor_tensor(i3a[:], m[:], -6.0, zxxx[:],
                                       ALU.mult, ALU.add)
                n_t = _t(tpool, "n")
                v.tensor_tensor(n_t[:], h6[:], zx3[:], ALU.mult)
                i3 = _t(tpool, "i3")
                v.tensor_tensor(i3[:], i3a[:], n_t[:], ALU.add)
                axxx_n = _t(spool, "axxx")
                g.tensor_tensor(axxx_n[:], f1[:], i3[:], ALU.mult)

                a, at, ax, axxx = a_n, at_n, ax_n, axxx_n
                if not last:
                    axx = axx_n

            a5, ax5, at5, axxx5 = a, ax, at, axxx

        if stage == "tower":
            nc.sync.dma_start(loss_d[:], axxx5[:].bitcast(F32))
            return

        # ---------- layer 6 projection + Gram ----------
        # U8 rows (2s+b): s=0 u, 1 uxxx, 2 ux, 3 ut; b = block.
        with tc.tile_pool(name="proj", bufs=2, space="PSUM") as ppool:
            # pair tiles [2,FD] fp16, rows = (block0, block1):
            # puux = (u+b6)*ux, puxxx, pux, put. All partition-0 based.
            # U2 slots rotate (bufs=2); order keeps WAR deps acyclic.
            gV = cpool.tile([8, FD], SDT, tag="gV")
            Ux = ppool.tile([2, FD], F32, tag="U2")
            _mm_chunks(nc, Ux, w6p[:], ax5[:])
            pux = cpool.tile([2, FD], SDT, tag="pux")
            s.activation(pux[:], Ux[:], AF.Copy)
            nc.scalar.dma_start(gV[4:6, :], pux[:])
            Uu = ppool.tile([2, FD], F32, tag="U2")
            _mm_chunks(nc, Uu, w6p[:], a5[:])
            v.scalar_tensor_tensor(gV[0:2, :], Uu[:], b6sc, pux[:],
                                   ALU.add, ALU.mult)
            Ut = ppool.tile([2, FD], F32, tag="U2")
            _mm_chunks(nc, Ut, w6p[:], at5[:])
            put = cpool.tile([2, FD], SDT, tag="put")
            s.activation(put[:], Ut[:], AF.Copy)
            nc.gpsimd.dma_start(gV[6:8, :], put[:])
            Uxxx = ppool.tile([2, FD], F32, tag="U2")
            _mm_chunks(nc, Uxxx, w6p[:], axxx5[:])
            puxxx = cpool.tile([2, FD], SDT, tag="puxxx")
            s.activation(puxxx[:], Uxxx[:], AF.Copy)
            nc.sync.dma_start(gV[2:4, :], puxxx[:])

        with tc.tile_pool(name="psmall", bufs=1, space="PSUM") as pps:
            G4 = pps.tile([4, 4], F32, tag="G4")
            TCH = ((0, 128), (128, 128), (256, 128), (384, 128), (512, 113))
            for c, (lo, w) in enumerate(TCH):
                chT_p = pps.tile([128, 8], SDT, tag="chT")
                nc.tensor.transpose(chT_p[0:w, :], gV[:, lo:lo + w], iden8[:])
                chT = tpool.tile([128, 8], SDT, tag="chTs")
                v.tensor_copy(chT[0:w, :], chT_p[0:w, :])
                chv = chT[0:w, :].rearrange("p (s b) -> p b s", b=2, s=4)
                for b in range(2):
                    _mm(nc, G4[:], chv[:, b, :], chv[:, b, :],
                        start=(c == 0 and b == 0), stop=(c == 4 and b == 1))

            g16f = cpool.tile([4, 4], SDT, tag="g16f")
            s.activation(g16f[:], G4[:], AF.Copy, scale=GS)

            if stage == "gram":
                gg = cpool.tile([4, 4], F32, tag="gg")
                v.tensor_copy(gg[:], G4[:])
                nc.sync.dma_start(loss_d[:], gg[:])
                return

            # bounce g16 through DRAM once, then DVE-replicate to the
            # 32-aligned diagonal blocks (start partitions 0/32/64/96).
            g16d = dpool.tile([1, 16], SDT, tag="g16d")
            nc.sync.dma_start(g16d[:], g16f[:])
            nc.sync.dma_start(gBD[0:16, 0:1], g16d[:])
            for bidx in range(1, PG):
                v.tensor_copy(gBD[32 * bidx:32 * bidx + 16, bidx:bidx + 1],
                              gBD[0:16, 0:1])

            # ---------- partial losses for all 5000 paras ----------
            loss8 = pps.tile([PG, PPG], F32, tag="loss8")
            PCH = ((0, 512), (512, 512), (1024, PPG - 1024))
            _mm_chunks(nc, loss8, gBD[:], mono[:], chunks=PCH)
            lossS = cpool.tile([PG, PPG], F32, tag="lossS")
            s.activation(lossS[:], loss8[:], AF.Copy)
            nc.sync.dma_start(loss_d[:], lossS[:])


def prep_inputs(x, para, W1, b1, W2, b2, W3, b3, W4, b4, W5, b5, W6, b6):
    """Full inputs -> list of per-core input dicts (host-side shard/layout)."""
    f = np.float32
    h = NPDT
    x = np.asarray(x, f)
    para = np.asarray(para, f)
    Ws = [np.asarray(W, f) for W in (W1, W2, W3, W4, W5, W6)]
    bs = [np.asarray(b, f) for b in (b1, b2, b3, b4, b5, b6)]

    w1t = np.zeros((4, HB), h)
    w1t[0:2, 0:50] = Ws[0].T
    w1t[2:4, 50:100] = Ws[0].T
    wb = np.zeros((HB, 400), h)
    for i in range(4):
        W = Ws[i + 1]
        wb[0:50, 100 * i:100 * i + 50] = W.T
        wb[50:100, 100 * i + 50:100 * i + 100] = W.T
    w6p = np.zeros((HB, 2), h)
    w6p[0:50, 0] = Ws[5][0]
    w6p[50:100, 1] = Ws[5][0]
    vecs = np.zeros((HB, 11), f)
    vecs[:, 10] = bs[5][0]
    vecs[:, 9] = -2.0
    cx = Ws[0][:, 0]
    ct = Ws[0][:, 1]
    for half in (slice(0, 50), slice(50, 100)):
        vecs[half, 0] = cx
        vecs[half, 1] = ct
        vecs[half, 2] = -2.0 * cx * cx
        vecs[half, 3] = cx * cx * cx
        for l in range(5):
            vecs[half, 4 + l] = bs[l]
    iden8 = np.eye(8, dtype=h)

    # mono[16*b + 4*i + j, k] = ptilde_i * ptilde_j * GS for para[625*b + k]
    pt = np.concatenate([para, np.ones((5000, 1), f)], axis=1)  # [5000,4]
    mono_full = (pt[:, :, None] * pt[:, None, :] * GS).reshape(5000, 16)
    mono = np.zeros((128, PPG), h)
    for b in range(PG):
        mono[32 * b:32 * b + 16, :] = mono_full[PPG * b:PPG * (b + 1)].T

    maps = []
    for c in range(NCORES):
        sl = x[c * NPC:(c + 1) * NPC]
        h0 = np.zeros((4, FD), h)
        h0[0] = sl[0:FD, 0]
        h0[1] = sl[0:FD, 1]
        h0[2] = sl[FD:NPC, 0]
        h0[3] = sl[FD:NPC, 1]
        maps.append({
            "h0": h0, "w1t": w1t, "wb": wb, "w6p": w6p, "vecs": vecs,
            "iden8": iden8, "mono": mono,
        })
    return maps


_NC_CACHE = {}


def get_program():
    if "nc" not in _NC_CACHE:
        _NC_CACHE["nc"] = build_program()
    return _NC_CACHE["nc"]


def kernel(x, para, W1, b1, W2, b2, W3, b3, W4, b4, W5, b5, W6, b6):
    maps = prep_inputs(x, para, W1, b1, W2, b2, W3, b3, W4, b4, W5, b5, W6, b6)
    nc = get_program()
    res = bass_utils.run_bass_kernel_spmd(nc, maps, list(range(NCORES)))
    out = np.zeros(5000, np.float64)
    for c in range(NCORES):
        out += res.results[c]["loss"].astype(np.float64).reshape(-1)
    return out.astype(np.float32)
